# revision 35
# baseline (speedup 1.0000x reference)
"""Trainium2 Bass kernel for: relu(1 - beta + x @ W^T).

Shapes (hardcoded): x [4096, 4096] f32, weights [4096, 4096] f32, beta [1] f32.
Output: [4096, 4096] f32.

Strategy: 8 cores as a 4 (batch) x 2 (output) grid. Host pre-transposes x/W to
fp16 so the contraction dim (IN) lands on SBUF partitions with contiguous DMA;
matmuls run fp16 x fp16 -> fp32 PSUM (~3e-4 rel err), the ReLU + (1-beta)
bias epilogue reads PSUM on ScalarE/VectorE. Raw Bacc (no Tile) with
hand-rolled semaphores.

v3 structure:
  - pass 0 streams w tiles fine-grained, kt-outer/m-inner. The startup
    critical tiles are spread across all three DGE queues so their ~4us
    DMA fixed costs overlap: w t0 chunk0 on sync, chunk1 on scalar, w t1
    on gpsimd -- first real matmul ~2us earlier than a single-queue chain.
  - passes 1..NT-1 read a fully-resident 512-col w column (two alternating
    SBUF buffers, loaded a full pass ahead of use and gated OUT of the
    startup window), looping m-outer/kt-inner so each m's epilogue + store
    pipeline behind the remaining matmuls.
  - last pass runs odd-m groups first, even-m last: the final group's
    epilogue is a scalar ACT with an inline HWDGE store, split in two
    256-col halves, so the post-matmul drain is ~1.5us.
  - no kernel-side teardown: the NEFF exit sequence drains every engine's
    DMA queues at the final barrier and then resets the (fixed-range)
    semaphore file; landing of the sem-free last stores is covered.

Engine roles:
  sync   -- pass-0 w loads, beta, pass>=2 w-column loads, last-pass odd-m
            stores
  scalar -- startup x lower halves + w t0 chunk1 + x t2/t3, pass-1 w column
            (delayed), even-m epilogues, inline last-pass even-m stores
  gpsimd -- w t1, bulk x loads (paced behind pass-0 w), mid-pass stores
  vector -- bias compute + odd-m epilogues
  tensor -- warmup + 1024 matmuls

Parameterized sizes so a miniature version can be validated in CoreSim.
"""
import numpy as np

import concourse.bass as bass
import concourse.mybir as mybir
from concourse import bacc

F32 = mybir.dt.float32
F16 = mybir.dt.float16
N_WARMUP_MM = 34   # dummy PE matmuls at stream start to warm the HAM clock
                   # while the first x/w DMAs land (~3.6us at 107ns each --
                   # the first 64KB DMA takes ~4.3us end-to-end)


def build_v3(IN=4096, MB=1024, NO=2048, safe_exit=False, n_warmup=N_WARMUP_MM,
             debug=False):
    KT = IN // 128          # contraction tiles
    NT = NO // 512          # output-col passes
    MT = MB // 128          # batch-row tiles (psum banks used)
    assert MT <= 8 and MT % 2 == 0 and NT >= 2 and KT >= 8

    # pass-0 w tile groups after the four singles: quad then sextets
    def groups_after4():
        gs = [(4, 4)]
        t = 8
        while t < KT:
            n = min(6, KT - t)
            gs.append((t, n))
            t += n
        return gs

    W_GROUPS = groups_after4()          # [(start, ntiles)]
    # x bulk loads: tiles 4-7 as two pairs (finer arrival deadlines than the
    # w quad -- x tiles are 2x the bytes), then the same sextets as w
    X_GROUPS = [(4, 2), (6, 2)] + W_GROUPS[1:]
    NCH = max(1, KT // 8)               # 8-tile chunks per w column load
    assert KT % NCH == 0
    CHT = KT // NCH                     # tiles per column chunk

    # last pass: odd-m groups first, even-m last (the final group's store
    # is a scalar-inline HWDGE store -- shortest possible drain)
    def pass_order(j):
        if j == NT - 1:
            return list(range(1, MT, 2)) + list(range(0, MT, 2))
        return list(range(MT))

    # position of group m within pass j's issue order
    def pos_of(j, m):
        return pass_order(j).index(m)

    nc = bacc.Bacc("TRN2", target_bir_lowering=False, debug=debug)
    xT = nc.dram_tensor("xT", [IN, MB], F16, kind="ExternalInput").ap()
    wT = nc.dram_tensor("wT", [IN, NO], F16, kind="ExternalInput").ap()
    beta = nc.dram_tensor("beta", [128, 1], F32, kind="ExternalInput").ap()
    out = nc.dram_tensor("out", [MB, NO], F16, kind="ExternalOutput").ap()

    x_sb = nc.alloc_sbuf_tensor("x_sb", [128, KT, MB], F16).ap()
    w0_sb = nc.alloc_sbuf_tensor("w0_sb", [128, KT, 512], F16).ap()
    wc_sb = nc.alloc_sbuf_tensor("wc_sb", [128, 2, KT, 512], F16).ap()
    o_sb = nc.alloc_sbuf_tensor("o_sb", [128, 2, MT, 512], F16).ap()
    beta_sb = nc.alloc_sbuf_tensor("beta_sb", [128, 1], F32).ap()
    bias_sb = nc.alloc_sbuf_tensor("bias_sb", [128, 1], F32).ap()
    ps = nc.alloc_psum_tensor("ps", [128, MT, 512], F32).ap()

    # ---- semaphores ----
    s_xs = [nc.alloc_semaphore(f"s_xs{k}") for k in range(2)]   # x t0/t1 lower
    s_xu = [nc.alloc_semaphore(f"s_xu{k}") for k in range(2)]   # x t0/t1 upper
    s_x23 = [nc.alloc_semaphore(f"s_x{k}") for k in (2, 3)]     # x t2/t3
    s_xg = [nc.alloc_semaphore(f"s_xg{i}") for i in range(len(X_GROUPS))]
    # w tiles 0-2 land in two 256-col chunks each so compute can start on
    # the first 64KB of every startup-critical tile
    s_wa = [nc.alloc_semaphore(f"s_w{k}a") for k in range(3)]
    s_wb = [nc.alloc_semaphore(f"s_w{k}b") for k in range(3)]
    s_w3 = nc.alloc_semaphore("s_w3")                           # w t3 (SWDGE)
    s_wg = [nc.alloc_semaphore(f"s_wg{i}") for i in range(len(W_GROUPS))]
    s_wc = [nc.alloc_semaphore(f"s_wc{b}") for b in range(2)]   # w column bufs
    s_b = nc.alloc_semaphore("s_b")                             # beta arrival
    s_bias = nc.alloc_semaphore("s_bias")                       # bias computed
    s_mm = nc.alloc_semaphore("s_mm")    # accum groups done, in issue order
    s_eps = nc.alloc_semaphore("s_eps")  # scalar epilogue ops (+1)
    s_epv = nc.alloc_semaphore("s_epv")  # vector epilogue ops (+1)
    s_o = [nc.alloc_semaphore("s_o0"), nc.alloc_semaphore("s_o1")]
    # last-pass store sems (never waited on; one per DGE queue -- SWDGE
    # sems can't be shared with HWDGE updates)
    s_oLh = nc.alloc_semaphore("s_oLh")

    # mid-pass store accounting (passes 0..NT-2, 2 DMAs x16 each)
    o_slot_cum = [0, 0]
    o_targets = []                       # cumulative per slot AFTER each pass
    for j in range(NT - 1):
        o_slot_cum[j % 2] += 32
        o_targets.append(o_slot_cum[j % 2])

    # epilogue inc target for (j, m): scalar does even m, vector odd; each
    # engine processes its m's in ascending order in every pass
    def ep_wait(j, m):
        if m % 2 == 0:
            return s_eps, (MT // 2) * j + m // 2 + 1
        return s_epv, (MT // 2) * j + (m - 1) // 2 + 1

    # s_mm target for "group (j, m) complete"
    def mm_target(j, m):
        return MT * j + pos_of(j, m) + 1

    # wc buffer + cumulative arrival target for pass j (j >= 1)
    def wc_of(j):
        buf = (j - 1) % 2
        gen = (j - 1) // 2 + 1
        return buf, 16 * NCH * gen

    def dma_rows(eng, dst, row0, ntiles, ncols, src, col0, sem):
        eng.dma_start(
            dst[:, row0:row0 + ntiles, :],
            src[row0 * 128:(row0 + ntiles) * 128,
                col0:col0 + ncols].rearrange("(k p) c -> p k c", p=128),
        ).then_inc(sem, 16)

    def wc_load(eng, buf, j):
        for c in range(NCH):
            eng.dma_start(
                wc_sb[:, buf, c * CHT:(c + 1) * CHT, :],
                wT[c * CHT * 128:(c + 1) * CHT * 128,
                   j * 512:(j + 1) * 512].rearrange("(k p) c -> p k c", p=128),
            ).then_inc(s_wc[buf], 16)

    with nc.Block() as block:

        @block.sync
        def _(sync: bass.BassEngine):
            # w-only at startup, strictly in deadline order; the singles
            # t1/t2/t3 go via the other queues so the quad + sextets are
            # nearly first in line here (beta isn't needed until pass-0 end)
            sync.dma_start(
                w0_sb[:, 0, 0:256], wT[0:128, 0:256],
            ).then_inc(s_wa[0], 16)
            for ci, sem in ((0, s_wa[2]), (1, s_wb[2])):
                sync.dma_start(
                    w0_sb[:, 2, ci * 256:(ci + 1) * 256],
                    wT[2 * 128:3 * 128, ci * 256:(ci + 1) * 256],
                ).then_inc(sem, 16)
            for gi, (t0, n) in enumerate(W_GROUPS):
                dma_rows(sync, w0_sb, t0, n, 512, wT, 0, s_wg[gi])
            sync.dma_start(beta_sb[:], beta[:]).then_inc(s_b, 16)
            # w columns for passes >= 2: gated on the pass two earlier being
            # consumed (buffer reuse for j>=3) and kept out of the startup
            # window (j==2)
            for j in range(2, NT):
                buf, _tgt = wc_of(j)
                sync.wait_ge(s_mm, MT * (j - 1))
                wc_load(sync, buf, j)
            # last pass: odd-m stores (processed first in the pass, so these
            # complete well before the final even groups)
            j = NT - 1
            for m in range(1, MT, 2):
                wsem, wval = ep_wait(j, m)
                sync.wait_ge(wsem, wval)
                sync.dma_start(
                    out[m * 128:(m + 1) * 128, j * 512:(j + 1) * 512],
                    o_sb[:, j % 2, m, :],
                ).then_inc(s_oLh, 16)

        @block.scalar
        def _(scalar: bass.BassEngine):
            half = MB // 2
            # startup on the ACT HWDGE queue: x lower halves (first matmuls'
            # rows), w t0 chunk1, w t2, x t2
            scalar.dma_start(
                x_sb[:, 0, 0:half], xT[0:128, 0:half],
            ).then_inc(s_xs[0], 16)
            scalar.dma_start(
                w0_sb[:, 0, 256:512], wT[0:128, 256:512],
            ).then_inc(s_wb[0], 16)
            scalar.dma_start(
                x_sb[:, 1, 0:half], xT[128:256, 0:half],
            ).then_inc(s_xs[1], 16)
            scalar.dma_start(
                x_sb[:, 2, :], xT[2 * 128:3 * 128, :],
            ).then_inc(s_x23[0], 16)
            # pass-1 w column: delayed until pass-0 w has landed so it stays
            # out of the startup window (needed complete only by pass-0 end)
            scalar.wait_ge(s_wg[len(W_GROUPS) - 1], 16)
            wc_load(scalar, 0, 1)
            for j in range(NT):
                for m in range(0, MT, 2):
                    scalar.wait_ge(s_mm, mm_target(j, m))
                    if j == 0 and m == 0:
                        scalar.wait_ge(s_bias, 1)
                    if j >= 2:
                        scalar.wait_ge(s_o[j % 2], o_targets[j - 2])
                    if j == NT - 1:
                        # split epilogue + inline store in two 256-col halves
                        # so the store starts as early as possible. Last-pass
                        # s_eps counts 2 per even m (nothing downstream waits
                        # on last-pass s_eps values); the wait before each
                        # store is trivially satisfied and only makes the
                        # ACT->DMA ordering explicit for the race detector.
                        # For the final even m the PE ran two half groups:
                        # half B lives in psum bank 0 and completes one
                        # s_mm later.
                        final_split = (m == pass_order(j)[-1] and MT >= 4)
                        base = (MT // 2) * j + (m // 2) * 2
                        for h in range(2):
                            if final_split and h == 1:
                                scalar.wait_ge(s_mm, MT * j + MT + 1)
                                src = ps[:, 0, 0:256]
                            else:
                                src = ps[:, m, h * 256:(h + 1) * 256]
                            scalar.activation(
                                o_sb[:, j % 2, m, h * 256:(h + 1) * 256],
                                src,
                                mybir.ActivationFunctionType.Relu,
                                bias=bias_sb[:], scale=1.0,
                            ).then_inc(s_eps, 1)
                            scalar.wait_ge(s_eps, base + h + 1)
                            scalar.dma_start(
                                out[m * 128:(m + 1) * 128,
                                    j * 512 + h * 256:j * 512 + (h + 1) * 256],
                                o_sb[:, j % 2, m, h * 256:(h + 1) * 256],
                            ).then_inc(s_oLh, 16)
                    else:
                        scalar.activation(
                            o_sb[:, j % 2, m, :], ps[:, m, :],
                            mybir.ActivationFunctionType.Relu,
                            bias=bias_sb[:], scale=1.0,
                        ).then_inc(s_eps, 1)

        @block.vector
        def _(vector: bass.BassEngine):
            vector.wait_ge(s_b, 16)
            vector.tensor_scalar(
                bias_sb[:], beta_sb[:], -1.0, -1.0,
                mybir.AluOpType.mult, mybir.AluOpType.subtract,
            ).then_inc(s_bias, 1)
            for j in range(NT):
                for m in range(1, MT, 2):
                    vector.wait_ge(s_mm, mm_target(j, m))
                    if j == 0 and m == 1:
                        # engine-ordered after the bias compute above; the
                        # wait is trivially satisfied (race-detector aid)
                        vector.wait_ge(s_bias, 1)
                    if j >= 2:
                        vector.wait_ge(s_o[j % 2], o_targets[j - 2])
                    vector.tensor_scalar(
                        o_sb[:, j % 2, m, :], ps[:, m, :], bias_sb[:], 0.0,
                        mybir.AluOpType.add, mybir.AluOpType.max,
                    ).then_inc(s_epv, 1)

        @block.gpsimd
        def _(gpsimd: bass.BassEngine):
            half = MB // 2
            # w t1 first on the SWDGE queue (startup-critical), then x
            # uppers, w t3, and the bulk x loads; x beyond tile 13 is paced
            # behind the first w sextet so the w tail keeps its bandwidth
            gpsimd.dma_start(
                w0_sb[:, 1, 0:256], wT[128:256, 0:256],
            ).then_inc(s_wa[1], 16)
            gpsimd.dma_start(
                x_sb[:, 0, half:MB], xT[0:128, half:MB],
            ).then_inc(s_xu[0], 16)
            gpsimd.dma_start(
                w0_sb[:, 1, 256:512], wT[128:256, 256:512],
            ).then_inc(s_wb[1], 16)
            gpsimd.dma_start(
                x_sb[:, 1, half:MB], xT[128:256, half:MB],
            ).then_inc(s_xu[1], 16)
            gpsimd.dma_start(
                w0_sb[:, 3, :], wT[3 * 128:4 * 128, 0:512],
            ).then_inc(s_w3, 16)
            gpsimd.dma_start(
                x_sb[:, 3, :], xT[3 * 128:4 * 128, :],
            ).then_inc(s_x23[1], 16)
            # pace the bulk x behind pass-0 w arrivals: with all 8 cores
            # bursting at once, ungated multi-MB x loads here collapse the
            # per-core HBM share and starve the startup-critical w tiles
            for gi, (t0, n) in enumerate(X_GROUPS):
                if gi == 1:
                    gpsimd.wait_ge(s_wb[2], 16)
                elif gi == 2:
                    gpsimd.wait_ge(s_wg[0], 16)
                elif gi == 3:
                    gpsimd.wait_ge(s_wg[1], 16)
                dma_rows(gpsimd, x_sb, t0, n, MB, xT, 0, s_xg[gi])
            # mid-pass stores (passes 0..NT-2): both 4-m halves as two DMAs
            h2 = MT // 2
            for j in range(NT - 1):
                gpsimd.wait_ge(s_eps, (MT // 2) * (j + 1))
                gpsimd.wait_ge(s_epv, (MT // 2) * (j + 1))
                for h in range(2):
                    gpsimd.dma_start(
                        out[h * h2 * 128:(h + 1) * h2 * 128,
                            j * 512:(j + 1) * 512].rearrange(
                                "(m p) c -> p m c", p=128),
                        o_sb[:, j % 2, h * h2:(h + 1) * h2, :],
                    ).then_inc(s_o[j % 2], 16)

        @block.tensor
        def _(tensor: bass.BassEngine):
            # Warm the PE clock (HAM) while the first x/w chunks land
            for _ in range(n_warmup):
                tensor.matmul(ps[:, 0, 0:128], x_sb[:, 0, 0:128],
                              w0_sb[:, 0, 0:128],
                              start=True, stop=True, skip_group_check=True)
            # ---- pass 0: kt-outer / m-inner over streamed w tiles ----
            wg_next = 0
            for kt in range(KT):
                if kt < 3:
                    tensor.wait_ge(s_wa[kt], 16)
                    for m in range(MT):
                        if kt < 2:
                            if m == 0:
                                tensor.wait_ge(s_xs[kt], 16)
                            elif m == MT // 2:
                                tensor.wait_ge(s_xu[kt], 16)
                        elif m == 0:
                            tensor.wait_ge(s_x23[0], 16)
                        tensor.matmul(
                            ps[:, m, 0:256],
                            x_sb[:, kt, m * 128:(m + 1) * 128],
                            w0_sb[:, kt, 0:256],
                            start=(kt == 0), stop=False,
                            skip_group_check=(kt != 0),
                        )
                        if m == 0:
                            tensor.wait_ge(s_wb[kt], 16)
                        tensor.matmul(
                            ps[:, m, 256:512],
                            x_sb[:, kt, m * 128:(m + 1) * 128],
                            w0_sb[:, kt, 256:512],
                            start=False, stop=False,
                            skip_group_check=True,
                        )
                    continue
                if kt == 3:
                    tensor.wait_ge(s_w3, 16)
                elif wg_next < len(W_GROUPS) and kt == W_GROUPS[wg_next][0]:
                    tensor.wait_ge(s_wg[wg_next], 16)
                    wg_next += 1
                for m in range(MT):
                    if m == 0:
                        if kt == 3:
                            tensor.wait_ge(s_x23[1], 16)
                        else:
                            for gi, (t0, n) in enumerate(X_GROUPS):
                                if kt == t0:
                                    tensor.wait_ge(s_xg[gi], 16)
                    mm = tensor.matmul(
                        ps[:, m, :],
                        x_sb[:, kt, m * 128:(m + 1) * 128],
                        w0_sb[:, kt, :],
                        start=False, stop=(kt == KT - 1),
                    )
                    if kt == KT - 1:
                        mm.then_inc(s_mm, 1)
            # ---- passes 1..NT-1: m-outer / kt-inner over resident columns
            for j in range(1, NT):
                buf, tgt = wc_of(j)
                first = True
                for m in pass_order(j):
                    if first:
                        tensor.wait_ge(s_wc[buf], tgt)
                        first = False
                    wsem, wval = ep_wait(j - 1, m)
                    tensor.wait_ge(wsem, wval)
                    if j == NT - 1 and m == pass_order(j)[-1] and MT >= 4:
                        # final group: two independent 256-col halves so the
                        # first half's epilogue + store overlap the second
                        # half's matmuls. Half B accumulates into bank 0
                        # (long free -- its last-pass epilogue, 2 s_eps incs,
                        # happened first among the evens), avoiding a
                        # PE-write/ACT-read collision on bank m.
                        for kt in range(KT):
                            mm = tensor.matmul(
                                ps[:, m, 0:256],
                                x_sb[:, kt, m * 128:(m + 1) * 128],
                                wc_sb[:, buf, kt, 0:256],
                                start=(kt == 0), stop=(kt == KT - 1),
                            )
                        mm.then_inc(s_mm, 1)
                        tensor.wait_ge(s_eps, (MT // 2) * (NT - 1) + 2)
                        for kt in range(KT):
                            mm = tensor.matmul(
                                ps[:, 0, 0:256],
                                x_sb[:, kt, m * 128:(m + 1) * 128],
                                wc_sb[:, buf, kt, 256:512],
                                start=(kt == 0), stop=(kt == KT - 1),
                            )
                        mm.then_inc(s_mm, 1)
                        continue
                    for kt in range(KT):
                        mm = tensor.matmul(
                            ps[:, m, :],
                            x_sb[:, kt, m * 128:(m + 1) * 128],
                            wc_sb[:, buf, kt, :],
                            start=(kt == 0), stop=(kt == KT - 1),
                        )
                    mm.then_inc(s_mm, 1)

    if safe_exit:
        # CoreSim runs: barrier so the race detector sees a quiescent end
        nc.sync.drain()
        nc.all_engine_barrier()
    nc.compile()
    return nc


GRID_B, GRID_O = 4, 2
MB_SHARD, NO_SHARD = 4096 // GRID_B, 4096 // GRID_O

_NC_CACHE = None


def _get_nc():
    global _NC_CACHE
    if _NC_CACHE is None:
        _NC_CACHE = build_v3(IN=4096, MB=MB_SHARD, NO=NO_SHARD)
    return _NC_CACHE


def kernel(x, weights, beta, _trace=False, _results_out=None):
    from concourse.bass_utils import run_bass_kernel_spmd

    x = np.asarray(x, dtype=np.float32)
    weights = np.asarray(weights, dtype=np.float32)
    beta = np.asarray(beta, dtype=np.float32)

    xT = np.ascontiguousarray(x.T.astype(np.float16))        # [IN, BATCH]
    wT = np.ascontiguousarray(weights.T.astype(np.float16))  # [IN, OUT]
    beta_b = np.ascontiguousarray(
        np.broadcast_to(beta.reshape(1, 1), (128, 1)).astype(np.float32)
    )

    in_maps = []
    for c in range(GRID_B * GRID_O):
        bi, oj = divmod(c, GRID_O)
        in_maps.append({
            "xT": np.ascontiguousarray(xT[:, bi * MB_SHARD:(bi + 1) * MB_SHARD]),
            "wT": np.ascontiguousarray(wT[:, oj * NO_SHARD:(oj + 1) * NO_SHARD]),
            "beta": beta_b,
        })

    nc = _get_nc()
    res = run_bass_kernel_spmd(
        nc, in_maps, core_ids=list(range(8)), trace=_trace,
        trace_cores=list(range(8)) if _trace else None,
    )
    if _results_out is not None:
        _results_out.append(res)

    out = np.empty((4096, 4096), dtype=np.float32)
    for c in range(GRID_B * GRID_O):
        bi, oj = divmod(c, GRID_O)
        out[bi * MB_SHARD:(bi + 1) * MB_SHARD,
            oj * NO_SHARD:(oj + 1) * NO_SHARD] = res.results[c]["out"]  # f16 -> f32
    return out


# revision 36
# speedup vs baseline: 1.0334x; 1.0334x over previous
"""Trainium2 Bass kernel for: relu(1 - beta + x @ W^T).

Shapes (hardcoded): x [4096, 4096] f32, weights [4096, 4096] f32, beta [1] f32.
Output: [4096, 4096] f32.

Strategy: 8 cores as a 4 (batch) x 2 (output) grid. Host pre-transposes x/W to
fp16 so the contraction dim (IN) lands on SBUF partitions with contiguous DMA;
matmuls run fp16 x fp16 -> fp32 PSUM (~3e-4 rel err), the ReLU + (1-beta)
bias epilogue reads PSUM on ScalarE/VectorE. Raw Bacc (no Tile) with
hand-rolled semaphores.

Structure (v8):
  - pass 0 streams w tiles fine-grained, kt-outer/m-inner. The startup
    critical tiles 0-2 land in two 256-col chunks each, spread across all
    three DGE queues so their ~4us DMA fixed costs overlap and compute can
    begin on the first 64KB. Bulk x loads are sem-gated behind pass-0 w
    arrivals: with 8 cores bursting at once, ungated multi-MB loads
    collapse the per-core HBM share (SDMA round-robins queues at packet
    granularity).
  - passes 1..NT-1 read a fully-resident 512-col w column (two alternating
    SBUF buffers, loaded a full pass ahead of use and gated OUT of the
    startup window), looping m-outer/kt-inner so each m's epilogue + store
    pipeline behind the remaining matmuls.
  - last pass runs odd-m groups first, even-m last: the final group's
    epilogue is a scalar ACT with an inline HWDGE store, split in two
    256-col halves, so the post-matmul drain is ~1.5us.
  - no kernel-side teardown: the NEFF exit sequence drains every engine's
    DMA queues at the final barrier and then resets the (fixed-range)
    semaphore file; landing of the sem-free last stores is covered.

Engine roles:
  sync   -- pass-0 w loads, beta, pass>=2 w-column loads, last-pass odd-m
            stores
  scalar -- startup x lower halves + w t0 chunk1 + x t2/t3, pass-1 w column
            (delayed), even-m epilogues, inline last-pass even-m stores
  gpsimd -- w t1, bulk x loads (paced behind pass-0 w), mid-pass stores
  vector -- bias compute + odd-m epilogues
  tensor -- warmup + 1024 matmuls

Parameterized sizes so a miniature version can be validated in CoreSim.
"""
import numpy as np

import concourse.bass as bass
import concourse.mybir as mybir
from concourse import bacc

F32 = mybir.dt.float32
F16 = mybir.dt.float16
N_WARMUP_MM = 34   # dummy PE matmuls at stream start to warm the HAM clock
                   # while the first x/w DMAs land (~3.6us at 107ns each --
                   # the first 64KB DMA takes ~4.3us end-to-end)


def build_v3(IN=4096, MB=1024, NO=2048, safe_exit=False, n_warmup=N_WARMUP_MM,
             debug=False):
    KT = IN // 128          # contraction tiles
    NT = NO // 512          # output-col passes
    MT = MB // 128          # batch-row tiles (psum banks used)
    assert MT <= 8 and MT % 2 == 0 and NT >= 2 and KT >= 8

    # pass-0 w tile groups after the four singles: quad then sextets
    def groups_after4():
        gs = [(4, 4)]
        t = 8
        while t < KT:
            n = min(6, KT - t)
            gs.append((t, n))
            t += n
        return gs

    W_GROUPS = groups_after4()          # [(start, ntiles)]
    # x bulk loads: tiles 4-7 as two pairs (finer arrival deadlines than the
    # w quad -- x tiles are 2x the bytes), then the same sextets as w
    X_GROUPS = [(4, 2), (6, 2)] + W_GROUPS[1:]
    NCH = max(1, KT // 8)               # 8-tile chunks per w column load
    assert KT % NCH == 0
    CHT = KT // NCH                     # tiles per column chunk

    # last pass: odd-m groups first, even-m last (the final group's store
    # is a scalar-inline HWDGE store -- shortest possible drain)
    def pass_order(j):
        if j == NT - 1:
            return list(range(1, MT, 2)) + list(range(0, MT, 2))
        return list(range(MT))

    # position of group m within pass j's issue order
    def pos_of(j, m):
        return pass_order(j).index(m)

    nc = bacc.Bacc("TRN2", target_bir_lowering=False, debug=debug)
    xT = nc.dram_tensor("xT", [IN, MB], F16, kind="ExternalInput").ap()
    wT = nc.dram_tensor("wT", [IN, NO], F16, kind="ExternalInput").ap()
    beta = nc.dram_tensor("beta", [128, 1], F32, kind="ExternalInput").ap()
    out = nc.dram_tensor("out", [MB, NO], F16, kind="ExternalOutput").ap()

    x_sb = nc.alloc_sbuf_tensor("x_sb", [128, KT, MB], F16).ap()
    w0_sb = nc.alloc_sbuf_tensor("w0_sb", [128, KT, 512], F16).ap()
    wc_sb = nc.alloc_sbuf_tensor("wc_sb", [128, 2, KT, 512], F16).ap()
    o_sb = nc.alloc_sbuf_tensor("o_sb", [128, 2, MT, 512], F16).ap()
    beta_sb = nc.alloc_sbuf_tensor("beta_sb", [128, 1], F32).ap()
    bias_sb = nc.alloc_sbuf_tensor("bias_sb", [128, 1], F32).ap()
    ps = nc.alloc_psum_tensor("ps", [128, MT, 512], F32).ap()

    # ---- semaphores ----
    s_xs = [nc.alloc_semaphore(f"s_xs{k}") for k in range(2)]   # x t0/t1 lower
    s_xu = [nc.alloc_semaphore(f"s_xu{k}") for k in range(2)]   # x t0/t1 upper
    s_x23 = [nc.alloc_semaphore(f"s_x{k}") for k in (2, 3)]     # x t2/t3
    s_xg = [nc.alloc_semaphore(f"s_xg{i}") for i in range(len(X_GROUPS))]
    # w tiles 0-2 land in two 256-col chunks each so compute can start on
    # the first 64KB of every startup-critical tile
    s_wa = [nc.alloc_semaphore(f"s_w{k}a") for k in range(3)]
    s_wb = [nc.alloc_semaphore(f"s_w{k}b") for k in range(3)]
    s_w3 = nc.alloc_semaphore("s_w3")                           # w t3 (SWDGE)
    s_wg = [nc.alloc_semaphore(f"s_wg{i}") for i in range(len(W_GROUPS))]
    s_wc = [nc.alloc_semaphore(f"s_wc{b}") for b in range(2)]   # w column bufs
    s_b = nc.alloc_semaphore("s_b")                             # beta arrival
    s_bias = nc.alloc_semaphore("s_bias")                       # bias computed
    s_mm = nc.alloc_semaphore("s_mm")    # accum groups done, in issue order
    s_eps = nc.alloc_semaphore("s_eps")  # scalar epilogue ops (+1)
    s_epv = nc.alloc_semaphore("s_epv")  # vector epilogue ops (+1)
    s_o = [nc.alloc_semaphore("s_o0"), nc.alloc_semaphore("s_o1")]
    # last-pass store sems (never waited on; one per DGE queue -- SWDGE
    # sems can't be shared with HWDGE updates)
    s_oLh = nc.alloc_semaphore("s_oLh")

    # mid-pass store accounting (passes 0..NT-2, 2 DMAs x16 each)
    o_slot_cum = [0, 0]
    o_targets = []                       # cumulative per slot AFTER each pass
    for j in range(NT - 1):
        o_slot_cum[j % 2] += 32
        o_targets.append(o_slot_cum[j % 2])

    # epilogue inc target for (j, m): scalar does even m, vector odd; each
    # engine processes its m's in ascending order in every pass
    def ep_wait(j, m):
        if m % 2 == 0:
            return s_eps, (MT // 2) * j + m // 2 + 1
        return s_epv, (MT // 2) * j + (m - 1) // 2 + 1

    # s_mm target for "group (j, m) complete"
    def mm_target(j, m):
        return MT * j + pos_of(j, m) + 1

    # wc buffer + cumulative arrival target for pass j (j >= 1)
    def wc_of(j):
        buf = (j - 1) % 2
        gen = (j - 1) // 2 + 1
        return buf, 16 * NCH * gen

    def dma_rows(eng, dst, row0, ntiles, ncols, src, col0, sem):
        eng.dma_start(
            dst[:, row0:row0 + ntiles, :],
            src[row0 * 128:(row0 + ntiles) * 128,
                col0:col0 + ncols].rearrange("(k p) c -> p k c", p=128),
        ).then_inc(sem, 16)

    def wc_load(eng, buf, j):
        for c in range(NCH):
            eng.dma_start(
                wc_sb[:, buf, c * CHT:(c + 1) * CHT, :],
                wT[c * CHT * 128:(c + 1) * CHT * 128,
                   j * 512:(j + 1) * 512].rearrange("(k p) c -> p k c", p=128),
            ).then_inc(s_wc[buf], 16)

    with nc.Block() as block:

        @block.sync
        def _(sync: bass.BassEngine):
            # w-only at startup, strictly in deadline order; the singles
            # t1/t2/t3 go via the other queues so the quad + sextets are
            # nearly first in line here (beta isn't needed until pass-0 end)
            sync.dma_start(
                w0_sb[:, 0, 0:256], wT[0:128, 0:256],
            ).then_inc(s_wa[0], 16)
            for ci, sem in ((0, s_wa[2]), (1, s_wb[2])):
                sync.dma_start(
                    w0_sb[:, 2, ci * 256:(ci + 1) * 256],
                    wT[2 * 128:3 * 128, ci * 256:(ci + 1) * 256],
                ).then_inc(sem, 16)
            for gi, (t0, n) in enumerate(W_GROUPS):
                dma_rows(sync, w0_sb, t0, n, 512, wT, 0, s_wg[gi])
            sync.dma_start(beta_sb[:], beta[:]).then_inc(s_b, 16)
            # w columns for passes >= 2: gated on the pass two earlier being
            # consumed (buffer reuse for j>=3) and kept out of the startup
            # window (j==2)
            for j in range(2, NT):
                buf, _tgt = wc_of(j)
                sync.wait_ge(s_mm, MT * (j - 1))
                wc_load(sync, buf, j)
            # last pass: odd-m stores (processed first in the pass, so these
            # complete well before the final even groups)
            j = NT - 1
            for m in range(1, MT, 2):
                wsem, wval = ep_wait(j, m)
                sync.wait_ge(wsem, wval)
                sync.dma_start(
                    out[m * 128:(m + 1) * 128, j * 512:(j + 1) * 512],
                    o_sb[:, j % 2, m, :],
                ).then_inc(s_oLh, 16)

        @block.scalar
        def _(scalar: bass.BassEngine):
            half = MB // 2
            # startup on the ACT HWDGE queue: x lower halves (first matmuls'
            # rows), w t0 chunk1, w t2, x t2
            scalar.dma_start(
                x_sb[:, 0, 0:half], xT[0:128, 0:half],
            ).then_inc(s_xs[0], 16)
            scalar.dma_start(
                w0_sb[:, 0, 256:512], wT[0:128, 256:512],
            ).then_inc(s_wb[0], 16)
            scalar.dma_start(
                x_sb[:, 1, 0:half], xT[128:256, 0:half],
            ).then_inc(s_xs[1], 16)
            scalar.dma_start(
                x_sb[:, 2, :], xT[2 * 128:3 * 128, :],
            ).then_inc(s_x23[0], 16)
            # pass-1 w column: delayed until pass-0 w has landed so it stays
            # out of the startup window (needed complete only by pass-0 end)
            scalar.wait_ge(s_wg[len(W_GROUPS) - 1], 16)
            wc_load(scalar, 0, 1)
            for j in range(NT):
                for m in range(0, MT, 2):
                    scalar.wait_ge(s_mm, mm_target(j, m))
                    if j == 0 and m == 0:
                        scalar.wait_ge(s_bias, 1)
                    if j >= 2:
                        scalar.wait_ge(s_o[j % 2], o_targets[j - 2])
                    if j == NT - 1:
                        # split epilogue + inline store in two 256-col halves
                        # so the store starts as early as possible. Last-pass
                        # s_eps counts 2 per even m (nothing downstream waits
                        # on last-pass s_eps values); the wait before each
                        # store is trivially satisfied and only makes the
                        # ACT->DMA ordering explicit for the race detector.
                        # For the final even m the PE ran two half groups:
                        # half B lives in psum bank 0 and completes one
                        # s_mm later.
                        final_split = (m == pass_order(j)[-1] and MT >= 4)
                        base = (MT // 2) * j + (m // 2) * 2
                        for h in range(2):
                            if final_split and h == 1:
                                scalar.wait_ge(s_mm, MT * j + MT + 1)
                                src = ps[:, 0, 0:256]
                            else:
                                src = ps[:, m, h * 256:(h + 1) * 256]
                            scalar.activation(
                                o_sb[:, j % 2, m, h * 256:(h + 1) * 256],
                                src,
                                mybir.ActivationFunctionType.Relu,
                                bias=bias_sb[:], scale=1.0,
                            ).then_inc(s_eps, 1)
                            scalar.wait_ge(s_eps, base + h + 1)
                            scalar.dma_start(
                                out[m * 128:(m + 1) * 128,
                                    j * 512 + h * 256:j * 512 + (h + 1) * 256],
                                o_sb[:, j % 2, m, h * 256:(h + 1) * 256],
                            ).then_inc(s_oLh, 16)
                    else:
                        scalar.activation(
                            o_sb[:, j % 2, m, :], ps[:, m, :],
                            mybir.ActivationFunctionType.Relu,
                            bias=bias_sb[:], scale=1.0,
                        ).then_inc(s_eps, 1)

        @block.vector
        def _(vector: bass.BassEngine):
            vector.wait_ge(s_b, 16)
            vector.tensor_scalar(
                bias_sb[:], beta_sb[:], -1.0, -1.0,
                mybir.AluOpType.mult, mybir.AluOpType.subtract,
            ).then_inc(s_bias, 1)
            for j in range(NT):
                for m in range(1, MT, 2):
                    vector.wait_ge(s_mm, mm_target(j, m))
                    if j == 0 and m == 1:
                        # engine-ordered after the bias compute above; the
                        # wait is trivially satisfied (race-detector aid)
                        vector.wait_ge(s_bias, 1)
                    if j >= 2:
                        vector.wait_ge(s_o[j % 2], o_targets[j - 2])
                    vector.tensor_scalar(
                        o_sb[:, j % 2, m, :], ps[:, m, :], bias_sb[:], 0.0,
                        mybir.AluOpType.add, mybir.AluOpType.max,
                    ).then_inc(s_epv, 1)

        @block.gpsimd
        def _(gpsimd: bass.BassEngine):
            half = MB // 2
            # w t1 first on the SWDGE queue (startup-critical), then x
            # uppers, w t3, and the bulk x loads; x beyond tile 13 is paced
            # behind the first w sextet so the w tail keeps its bandwidth
            gpsimd.dma_start(
                w0_sb[:, 1, 0:256], wT[128:256, 0:256],
            ).then_inc(s_wa[1], 16)
            gpsimd.dma_start(
                x_sb[:, 0, half:MB], xT[0:128, half:MB],
            ).then_inc(s_xu[0], 16)
            gpsimd.dma_start(
                w0_sb[:, 1, 256:512], wT[128:256, 256:512],
            ).then_inc(s_wb[1], 16)
            gpsimd.dma_start(
                x_sb[:, 1, half:MB], xT[128:256, half:MB],
            ).then_inc(s_xu[1], 16)
            gpsimd.dma_start(
                w0_sb[:, 3, :], wT[3 * 128:4 * 128, 0:512],
            ).then_inc(s_w3, 16)
            gpsimd.dma_start(
                x_sb[:, 3, :], xT[3 * 128:4 * 128, :],
            ).then_inc(s_x23[1], 16)
            # pace the bulk x behind pass-0 w arrivals: with all 8 cores
            # bursting at once, ungated multi-MB x loads here collapse the
            # per-core HBM share and starve the startup-critical w tiles
            for gi, (t0, n) in enumerate(X_GROUPS):
                if gi == 1:
                    gpsimd.wait_ge(s_wb[2], 16)
                elif gi == 2:
                    gpsimd.wait_ge(s_wg[0], 16)
                elif gi == 3:
                    gpsimd.wait_ge(s_wg[1], 16)
                dma_rows(gpsimd, x_sb, t0, n, MB, xT, 0, s_xg[gi])
            # mid-pass stores (passes 0..NT-2): both 4-m halves as two DMAs
            h2 = MT // 2
            for j in range(NT - 1):
                gpsimd.wait_ge(s_eps, (MT // 2) * (j + 1))
                gpsimd.wait_ge(s_epv, (MT // 2) * (j + 1))
                for h in range(2):
                    gpsimd.dma_start(
                        out[h * h2 * 128:(h + 1) * h2 * 128,
                            j * 512:(j + 1) * 512].rearrange(
                                "(m p) c -> p m c", p=128),
                        o_sb[:, j % 2, h * h2:(h + 1) * h2, :],
                    ).then_inc(s_o[j % 2], 16)

        @block.tensor
        def _(tensor: bass.BassEngine):
            # Warm the PE clock (HAM) while the first x/w chunks land
            for _ in range(n_warmup):
                tensor.matmul(ps[:, 0, 0:128], x_sb[:, 0, 0:128],
                              w0_sb[:, 0, 0:128],
                              start=True, stop=True, skip_group_check=True)
            # ---- pass 0: kt-outer / m-inner over streamed w tiles ----
            wg_next = 0
            for kt in range(KT):
                if kt < 3:
                    tensor.wait_ge(s_wa[kt], 16)
                    for m in range(MT):
                        if kt < 2:
                            if m == 0:
                                tensor.wait_ge(s_xs[kt], 16)
                            elif m == MT // 2:
                                tensor.wait_ge(s_xu[kt], 16)
                        elif m == 0:
                            tensor.wait_ge(s_x23[0], 16)
                        tensor.matmul(
                            ps[:, m, 0:256],
                            x_sb[:, kt, m * 128:(m + 1) * 128],
                            w0_sb[:, kt, 0:256],
                            start=(kt == 0), stop=False,
                            skip_group_check=(kt != 0),
                        )
                        if m == 0:
                            tensor.wait_ge(s_wb[kt], 16)
                        tensor.matmul(
                            ps[:, m, 256:512],
                            x_sb[:, kt, m * 128:(m + 1) * 128],
                            w0_sb[:, kt, 256:512],
                            start=False, stop=False,
                            skip_group_check=True,
                        )
                    continue
                if kt == 3:
                    tensor.wait_ge(s_w3, 16)
                elif wg_next < len(W_GROUPS) and kt == W_GROUPS[wg_next][0]:
                    tensor.wait_ge(s_wg[wg_next], 16)
                    wg_next += 1
                for m in range(MT):
                    if m == 0:
                        if kt == 3:
                            tensor.wait_ge(s_x23[1], 16)
                        else:
                            for gi, (t0, n) in enumerate(X_GROUPS):
                                if kt == t0:
                                    tensor.wait_ge(s_xg[gi], 16)
                    mm = tensor.matmul(
                        ps[:, m, :],
                        x_sb[:, kt, m * 128:(m + 1) * 128],
                        w0_sb[:, kt, :],
                        start=False, stop=(kt == KT - 1),
                    )
                    if kt == KT - 1:
                        mm.then_inc(s_mm, 1)
            # ---- passes 1..NT-1: m-outer / kt-inner over resident columns
            for j in range(1, NT):
                buf, tgt = wc_of(j)
                first = True
                for m in pass_order(j):
                    if first:
                        tensor.wait_ge(s_wc[buf], tgt)
                        first = False
                    wsem, wval = ep_wait(j - 1, m)
                    tensor.wait_ge(wsem, wval)
                    if j == NT - 1 and m == pass_order(j)[-1] and MT >= 4:
                        # final group: two independent 256-col halves so the
                        # first half's epilogue + store overlap the second
                        # half's matmuls. Half B accumulates into bank 0
                        # (long free -- its last-pass epilogue, 2 s_eps incs,
                        # happened first among the evens), avoiding a
                        # PE-write/ACT-read collision on bank m.
                        for kt in range(KT):
                            mm = tensor.matmul(
                                ps[:, m, 0:256],
                                x_sb[:, kt, m * 128:(m + 1) * 128],
                                wc_sb[:, buf, kt, 0:256],
                                start=(kt == 0), stop=(kt == KT - 1),
                            )
                        mm.then_inc(s_mm, 1)
                        tensor.wait_ge(s_eps, (MT // 2) * (NT - 1) + 2)
                        for kt in range(KT):
                            mm = tensor.matmul(
                                ps[:, 0, 0:256],
                                x_sb[:, kt, m * 128:(m + 1) * 128],
                                wc_sb[:, buf, kt, 256:512],
                                start=(kt == 0), stop=(kt == KT - 1),
                            )
                        mm.then_inc(s_mm, 1)
                        continue
                    for kt in range(KT):
                        mm = tensor.matmul(
                            ps[:, m, :],
                            x_sb[:, kt, m * 128:(m + 1) * 128],
                            wc_sb[:, buf, kt, :],
                            start=(kt == 0), stop=(kt == KT - 1),
                        )
                    mm.then_inc(s_mm, 1)

    if safe_exit:
        # CoreSim runs: barrier so the race detector sees a quiescent end
        nc.sync.drain()
        nc.all_engine_barrier()
    nc.compile()
    return nc


GRID_B, GRID_O = 4, 2
MB_SHARD, NO_SHARD = 4096 // GRID_B, 4096 // GRID_O

_NC_CACHE = None


def _get_nc():
    global _NC_CACHE
    if _NC_CACHE is None:
        _NC_CACHE = build_v3(IN=4096, MB=MB_SHARD, NO=NO_SHARD)
    return _NC_CACHE


def kernel(x, weights, beta, _trace=False, _results_out=None):
    from concourse.bass_utils import run_bass_kernel_spmd

    x = np.asarray(x, dtype=np.float32)
    weights = np.asarray(weights, dtype=np.float32)
    beta = np.asarray(beta, dtype=np.float32)

    xT = np.ascontiguousarray(x.T.astype(np.float16))        # [IN, BATCH]
    wT = np.ascontiguousarray(weights.T.astype(np.float16))  # [IN, OUT]
    beta_b = np.ascontiguousarray(
        np.broadcast_to(beta.reshape(1, 1), (128, 1)).astype(np.float32)
    )

    in_maps = []
    for c in range(GRID_B * GRID_O):
        bi, oj = divmod(c, GRID_O)
        in_maps.append({
            "xT": np.ascontiguousarray(xT[:, bi * MB_SHARD:(bi + 1) * MB_SHARD]),
            "wT": np.ascontiguousarray(wT[:, oj * NO_SHARD:(oj + 1) * NO_SHARD]),
            "beta": beta_b,
        })

    nc = _get_nc()
    res = run_bass_kernel_spmd(
        nc, in_maps, core_ids=list(range(8)), trace=_trace,
        trace_cores=list(range(8)) if _trace else None,
    )
    if _results_out is not None:
        _results_out.append(res)

    out = np.empty((4096, 4096), dtype=np.float32)
    for c in range(GRID_B * GRID_O):
        bi, oj = divmod(c, GRID_O)
        out[bi * MB_SHARD:(bi + 1) * MB_SHARD,
            oj * NO_SHARD:(oj + 1) * NO_SHARD] = res.results[c]["out"]  # f16 -> f32
    return out
